# revision 1
# baseline (speedup 1.0000x reference)
"""Trainium2 Bass kernel for nn_CrossAttention (sparse_attention).

Computes, for H=8 heads (one head per NeuronCore):
  q_g = (emb_g @ W_q + b_q)  per head   (g in {1,2})
  k_g = (emb_g @ W_k + b_k)  per head
  a_1[h] = (q_1[h] @ k_2[h]^T) * SCALE * mask_1     mask_1[i,j] = nt1[i]==nt2[j]
  a_2[h] = (k_1[h] @ q_2[h]^T) * SCALE * mask_2     mask_2 = mask_1^T
  out = concat([a_1, a_2]) -> [16, 2048, 2048]

Strategy: tensor-parallel over heads (core h owns head h and writes the
[2, N, N] slab). Since the mask is a node-type equality over only 5 types,
sorting both graphs' nodes by type (host-side permutation) makes each masked
score matrix block-diagonal: only the 5 matching-type blocks are nonzero.
The device computes just those blocks (5x fewer score FLOPs, no elementwise
mask work at all) and fills the rest of the output with DMA'd zeros; the
host scatters rows/cols back to the original order.
"""

import os
import numpy as np

N = 2048
D = 256
H = 8
T = 5
SCALE = D ** (-0.5)
NCORES = 8
P = 128

# float32r streams 1 col/cycle through the PE (vs 4 for float32) at reduced
# multiply precision. Toggled via env for A/B testing.
USE_F32R = os.environ.get("K_F32R", "1") == "1"
# The SPMD runner donates pre-zeroed output buffers (both the native and the
# PJRT path guarantee zero-initialized ExternalOutputs), so the off-block
# regions don't need explicit zero DMAs. K_ZEROS=1 restores them.
WRITE_ZEROS = os.environ.get("K_ZEROS", "0") == "1"

_PROG_CACHE: dict = {}


def _build_program(c1: tuple, c2: tuple, use_f32r: bool, write_zeros: bool):
    """Build + compile the per-core Bass program.

    c1/c2: per-type node counts for graph1/graph2 (segment sizes after the
    host-side stable sort by type). These are baked into matmul/DMA shapes.
    """
    import concourse.bass as bass  # noqa: F401
    import concourse.mybir as mybir
    import concourse.tile as tile
    from concourse import bacc
    from concourse.masks import make_identity

    f32 = mybir.dt.float32
    f32r = mybir.dt.float32r
    AF = mybir.ActivationFunctionType
    # dtype of matmul operand tiles; float32r operands must be *produced*
    # rounded (the BIR verifier enforces producer-side rounding), so the
    # PSUM->SBUF copies / activations write directly into f32r tiles.
    mdt = f32r if use_f32r else f32

    nc = bacc.Bacc("TRN2", target_bir_lowering=False, debug=False,
                   num_devices=NCORES)

    e_dram = {
        v: nc.dram_tensor(f"e_{v}", [N, D], f32, kind="ExternalInput")
        for v in ("q1", "k1", "k2", "q2")
    }
    wq_d = nc.dram_tensor("wq", [D, D], f32, kind="ExternalInput")
    wk_d = nc.dram_tensor("wk", [D, D], f32, kind="ExternalInput")
    bq_d = nc.dram_tensor("bq", [D], f32, kind="ExternalInput")
    bk_d = nc.dram_tensor("bk", [D], f32, kind="ExternalInput")
    out_d = nc.dram_tensor("out", [2, N, N], f32, kind="ExternalOutput")

    G = N // P          # 16 row tiles per emb
    C = D // P          # 2 contraction chunks

    # segment bounds
    def bounds(cnt):
        b = [0]
        for c in cnt:
            b.append(b[-1] + int(c))
        return b

    b1 = bounds(c1)
    b2 = bounds(c2)

    with tile.TileContext(nc) as tc:
        with (
            tc.tile_pool(name="const", bufs=1) as constp,
            tc.tile_pool(name="raw", bufs=2) as rawp,
            tc.tile_pool(name="embT", bufs=2) as embTp,
            tc.tile_pool(name="proj", bufs=1) as projp,
            tc.tile_pool(name="stage", bufs=16) as stagep,
            tc.tile_pool(name="ptp", bufs=2, space="PSUM") as psum_tp,
            tc.tile_pool(name="ppr", bufs=2, space="PSUM") as psum_pr,
            tc.tile_pool(name="pmm", bufs=2, space="PSUM") as psum_mm,
        ):
            ident = constp.tile([P, P], f32, tag="ident")
            make_identity(nc, ident[:])

            zero = None
            if write_zeros:
                zero = constp.tile([P, N], f32, tag="zero")
                nc.gpsimd.memset(zero[:], 0.0)

                # Zero-fill the off-block regions up front: these DMAs have no
                # compute dependencies, so they stream on the SP ring from t=0.
                def emit_zeros(mat, rb, cb):
                    for t in range(T):
                        c0, c1_ = cb[t], cb[t + 1]
                        for r0 in range(rb[t], rb[t + 1], P):
                            rows = min(P, rb[t + 1] - r0)
                            if c0 > 0:
                                nc.sync.dma_start(
                                    out_d[mat, r0:r0 + rows, 0:c0],
                                    zero[0:rows, 0:c0],
                                )
                            if c1_ < N:
                                nc.sync.dma_start(
                                    out_d[mat, r0:r0 + rows, c1_:N],
                                    zero[0:rows, 0:N - c1_],
                                )

                emit_zeros(0, b1, b2)
                emit_zeros(1, b2, b1)

            # weights: two row-chunk tiles [128, 256] per W. DMA as f32,
            # then a conversion copy into the matmul dtype (rounds for f32r).
            w_sb = {}
            for nm, dram in (("wq", wq_d), ("wk", wk_d)):
                for c in range(C):
                    t = constp.tile([P, D], f32, tag=f"{nm}{c}raw", name=f"{nm}{c}raw")
                    nc.sync.dma_start(t[:], dram[c * P:(c + 1) * P, :])
                    if use_f32r:
                        tm = constp.tile([P, D], mdt, tag=f"{nm}{c}", name=f"{nm}{c}m")
                        nc.vector.tensor_copy(tm[:], t[:])
                        w_sb[(nm, c)] = tm
                    else:
                        w_sb[(nm, c)] = t

            # biases as per-partition columns: [128, 2]; chunk c in column c
            b_sb = {}
            for nm, dram in (("bq", bq_d), ("bk", bk_d)):
                t = constp.tile([P, C], f32, tag=nm)
                nc.sync.dma_start(t[:], dram.ap().rearrange("(c p) -> p c", p=P))
                b_sb[nm] = t
            bq_s = constp.tile([P, C], f32, tag="bqs")
            nc.vector.tensor_scalar_mul(bq_s[:], b_sb["bq"][:], SCALE)

            # per-version: load -> transpose -> project
            pT = {}
            cp = 0  # copy-engine round robin

            def do_version(v, wname, qside):
                nonlocal cp
                raw = rawp.tile([P, G, D], f32, tag="raw", name=f"raw_{v}")
                # first version loads chunked so transposes start early;
                # later versions load whole (fewer DMAs). Alternate rings.
                e_re = e_dram[v].ap().rearrange("(g p) d -> p g d", p=P)
                for gc in range(0, G, 4):
                    nc.sync.dma_start(raw[:, gc:gc + 4, :], e_re[:, gc:gc + 4, :])
                eT = [embTp.tile([P, N], mdt, tag=f"eT{c}", name=f"eT_{v}_{c}")
                      for c in range(C)]
                # 4 [128,128] PE transposes share one PSUM bank -> 1 copy
                for c in range(C):
                    for g4 in range(0, G, 4):
                        ps = psum_tp.tile([P, 512], f32, tag="tp")
                        for gg in range(4):
                            nc.tensor.transpose(
                                ps[:, gg * P:(gg + 1) * P],
                                raw[:, g4 + gg, c * P:(c + 1) * P],
                                ident[:],
                            )
                        dst = eT[c][:, g4 * P:(g4 + 4) * P]
                        if cp % 2 == 0:
                            nc.vector.tensor_copy(dst, ps[:])
                        else:
                            nc.scalar.copy(dst, ps[:])
                        cp += 1

                pts = [projp.tile([P, N], mdt, tag=f"pT_{v}_{m}", name=f"pT_{v}_{m}")
                       for m in range(C)]
                for m in range(C):
                    for j2 in range(N // 1024):
                        ps = psum_pr.tile([P, 1024], f32, tag="pr", name="pr")
                        for jj in range(2):
                            j = j2 * 2 + jj
                            for c in range(C):
                                nc.tensor.matmul(
                                    ps[:, jj * 512:(jj + 1) * 512],
                                    w_sb[(wname, c)][:, m * P:(m + 1) * P],
                                    eT[c][:, j * 512:(j + 1) * 512],
                                    start=(c == 0),
                                    stop=(c == C - 1),
                                )
                        dst = pts[m][:, j2 * 1024:(j2 + 1) * 1024]
                        if qside:
                            nc.scalar.activation(
                                dst, ps[:], AF.Identity,
                                bias=bq_s[:, m:m + 1], scale=SCALE,
                            )
                        else:
                            nc.scalar.activation(
                                dst, ps[:], AF.Identity,
                                bias=b_sb["bk"][:, m:m + 1], scale=1.0,
                            )
                pT[v] = pts

            # scores: block-diagonal in sorted coordinates
            def do_matrix(mat, rT, cT, rb, cb, act_share):
                nonlocal cp
                for t in range(T):
                    r0s, r1s = rb[t], rb[t + 1]
                    c0, c1_ = cb[t], cb[t + 1]
                    for r0 in range(r0s, r1s, P):
                        r1 = min(r0 + P, r1s)
                        rows = r1 - r0
                        for j0 in range(c0, c1_, 512):
                            j1 = min(j0 + 512, c1_)
                            w = j1 - j0
                            # fp32r matmul needs an even moving-dim width and
                            # even PSUM width; pad within the pT buffer.
                            j0p, j1p = j0, j1
                            if use_f32r and w % 2 == 1:
                                if j1p < N:
                                    j1p += 1
                                else:
                                    j0p -= 1
                            wp = j1p - j0p
                            off = j0 - j0p
                            ps = psum_mm.tile([P, 512], f32, tag="mm", name="mm")
                            for c in range(C):
                                nc.tensor.matmul(
                                    ps[0:rows, 0:wp],
                                    rT[c][:, r0:r1],
                                    cT[c][:, j0p:j1p],
                                    start=(c == 0),
                                    stop=(c == C - 1),
                                )
                            st = stagep.tile([P, 512], f32, tag="st", name="st")
                            # pair the output DMA's ring with the copy's
                            # engine: a DMA that waits on its copy never
                            # head-of-line-blocks the other ring.
                            if cp % 2 == 0:
                                nc.vector.tensor_copy(
                                    st[0:rows, 0:w], ps[0:rows, off:off + w]
                                )
                                nc.sync.dma_start(
                                    out_d[mat, r0:r1, j0:j1], st[0:rows, 0:w]
                                )
                            else:
                                nc.scalar.copy(
                                    st[0:rows, 0:w], ps[0:rows, off:off + w]
                                )
                                nc.scalar.dma_start(
                                    out_d[mat, r0:r1, j0:j1], st[0:rows, 0:w]
                                )
                            cp += 1

            # a1 block t: q1[S1_t] @ k2[S2_t]^T ; a2 block t: k1[S2_t] @ q2[S1_t]^T
            # Interleave: a_1 scores run while k1/q2 still transpose/project,
            # so copy/DMA work reaches ACT/DVE/the wire early.
            do_version("q1", "wq", True)
            do_version("k2", "wk", False)
            do_matrix(0, pT["q1"], pT["k2"], b1, b2, act_share=True)
            do_version("k1", "wk", False)
            do_version("q2", "wq", True)
            do_matrix(1, pT["k1"], pT["q2"], b2, b1, act_share=True)

    nc.compile()
    return nc


def _get_program(c1, c2, use_f32r, write_zeros=WRITE_ZEROS):
    key = (tuple(c1), tuple(c2), use_f32r, write_zeros)
    if key not in _PROG_CACHE:
        _PROG_CACHE[key] = _build_program(key[0], key[1], use_f32r, write_zeros)
    return _PROG_CACHE[key]


def kernel(emb_1, emb_2, node_type_1, node_type_2, W_q, b_q, W_k, b_k):
    from concourse.bass_utils import run_bass_kernel_spmd

    emb_1 = np.ascontiguousarray(np.asarray(emb_1, dtype=np.float32))
    emb_2 = np.ascontiguousarray(np.asarray(emb_2, dtype=np.float32))
    nt1 = np.asarray(node_type_1).astype(np.int64)
    nt2 = np.asarray(node_type_2).astype(np.int64)
    W_q = np.asarray(W_q, dtype=np.float32)
    W_k = np.asarray(W_k, dtype=np.float32)
    b_q = np.asarray(b_q, dtype=np.float32)
    b_k = np.asarray(b_k, dtype=np.float32)

    perm1 = np.argsort(nt1, kind="stable")
    perm2 = np.argsort(nt2, kind="stable")
    c1 = np.bincount(nt1, minlength=T)
    c2 = np.bincount(nt2, minlength=T)

    e_q1 = np.ascontiguousarray(emb_1[perm1])   # q1 rows sorted by nt1
    e_k1 = np.ascontiguousarray(emb_1[perm2])   # k1 rows sorted by nt2 (mask_2 = mask_1.T)
    e_k2 = np.ascontiguousarray(emb_2[perm2])   # k2 cols sorted by nt2
    e_q2 = np.ascontiguousarray(emb_2[perm1])   # q2 cols sorted by nt1

    nc = _get_program(c1, c2, USE_F32R)

    in_maps = []
    for h in range(NCORES):
        sl = slice(h * D, (h + 1) * D)
        in_maps.append({
            "e_q1": e_q1,
            "e_k1": e_k1,
            "e_k2": e_k2,
            "e_q2": e_q2,
            "wq": np.ascontiguousarray(W_q[:, sl]),
            "wk": np.ascontiguousarray(W_k[:, sl]),
            "bq": np.ascontiguousarray(b_q[sl]),
            "bk": np.ascontiguousarray(b_k[sl]),
        })

    res = run_bass_kernel_spmd(nc, in_maps, core_ids=list(range(NCORES)))

    out = np.empty((2 * H, N, N), dtype=np.float32)
    r1 = perm1[:, None]
    r2 = perm2[:, None]
    col1 = perm1[None, :]
    col2 = perm2[None, :]
    for h in range(NCORES):
        slab = res.results[h]["out"]
        out[h][r1, col2] = slab[0]
        out[H + h][r2, col1] = slab[1]
    return out

